# revision 60
# baseline (speedup 1.0000x reference)
"""Trainium2 Bass kernel for causal multi-head attention.

Problem: B=4, T=2048, D=1024, H=16, HD=64, fp32, causal, scale=1/sqrt(D).

Sharding: 4-way batch x 2-way head-group over 8 cores. Core c=(b,g) computes
heads g*8..g*8+7 for batch b and returns the partial output projection
(contracted over its 512 context columns); the host sums the two partials
per batch element and adds bo.

Per-core dataflow:
  - Q/K/V projections run in fp8e4m3 DoubleRow mode (0.5 cycles/row, 256-deep
    contraction per instruction) using a hi+lo residual split of both the
    inputs and the weights: x@W ~= (xh+xl)@Wh + xh@Wl, which keeps the
    projection error at bf16 level while running ~2.7x faster than bf16.
    Weights are host-scaled by 32 so the fp8 residual stays in normal range
    (W ~ N(0,1/D) residuals are subnormal in e4m3 otherwise); 1/32 is folded
    back into the copyback / normalize constants. Host passes x^T pre-split
    into fp8 hi/lo pairs, so no on-device transposes or casts are needed.
  - Scores are computed transposed, S^T[tk_part, tq_free] (lhsT = K^T
    block, rhs = Q^T tile), in bf16; blocks above the causal diagonal are
    skipped entirely. Masking costs the PE nothing: each PV chain overlaps
    exactly one diagonal block's invalid triangle, which is zeroed by a
    single in-place DVE multiply inside the chain (its exp finished a head
    earlier, so the multiply never stalls anything).
  - Score blocks are exp'd in PAIRS: two 128x512 PSUM banks per scalar-engine
    instruction, halving the per-op overhead. The second slab of a diagonal
    pair reads some never-written PSUM lanes; the resulting garbage probs
    land in pt regions no consumer references.
  - PV runs in the SWAPPED orientation: out[tq,65] = pt_block^T @ VA,
    accumulated over key blocks (65-row matmuls are ~2x cheaper than the
    [65,tq] orientation and the tensor engine loads pt blocks as stationary
    weights for free). V is stored bf16 with a ones-column appended per head
    so the same accumulation yields the softmax denominator per PARTITION,
    which makes normalization a per-partition reciprocal + tensor_scalar on
    DVE - no partition-broadcast needed. The normalized [tok,128] tile of a
    HEAD PAIR is DMA-transposed (XBAR) back into the resident ctx^T buffer.
  - All PE work beyond the score stream (PV chains of the previous head,
    fp8 projection groups for the next tile, deferred output projections)
    lives in a FIFO work queue drained between score pairs, so the PE stays
    busy while the scalar engine paces the exp stream. x tiles are
    prefetched a full tq tile ahead; output projections are deferred two
    tiles so the ACT-bound last tile has PE filler work.
"""

import numpy as np
from contextlib import ExitStack

import ml_dtypes
import concourse.bass as bass
import concourse.tile as tile
from concourse import bacc
from concourse import mybir
from concourse.bass_utils import run_bass_kernel_spmd

F32 = mybir.dt.float32
F32R = mybir.dt.float32r
BF16 = mybir.dt.bfloat16
FP8 = mybir.dt.float8e4
AF = mybir.ActivationFunctionType
OP = mybir.AluOpType
DR = mybir.MatmulPerfMode.DoubleRow


def build_mha_core(T, D, F, DOUT, HD=64, TQ=512, scale=1.0, num_devices=1):
    """Build the per-core Bass program.

    T: tokens, D: model dim, F: feature columns owned by this core,
    DOUT: output projection width, HD: head dim, TQ: tq tile width.
    """
    NH = F // HD        # local heads
    DT = D // 128       # contraction tiles for projections
    DP = DT // 2        # DoubleRow pair count
    FT = F // 128       # feature 128-tiles
    NTOK = T // 128     # token 128-tiles
    NTQ = T // TQ       # tq tiles
    NR = TQ // 128      # 128-blocks per tq tile
    NCH = min(512, DOUT)
    NO = DOUT // NCH
    HPF = 128 // HD     # heads per feature tile

    nc = bacc.Bacc(None, target_bir_lowering=False, debug=False, num_devices=num_devices)

    xdr = {}
    for nm in ("q", "k", "v"):
        for half in ("h", "l"):
            xdr[nm + half] = nc.dram_tensor(f"{nm}T{half}", [D, T], FP8, kind="ExternalInput")
    wdr = {}
    for nm in ("q", "k", "v"):
        for half in ("h", "l"):
            wdr[nm + half] = nc.dram_tensor(f"W{nm}{half}", [D, F], FP8, kind="ExternalInput")
    Wo = nc.dram_tensor("Wo", [F, DOUT], BF16, kind="ExternalInput")
    bq = nc.dram_tensor("bq", [128, FT], F32, kind="ExternalInput")
    bk = nc.dram_tensor("bk", [128, FT], F32, kind="ExternalInput")
    bv = nc.dram_tensor("bv", [1, F], F32, kind="ExternalInput")
    ones = nc.dram_tensor("ones", [1, 1], BF16, kind="ExternalInput")
    mask01 = nc.dram_tensor("mask01", [128, 128], BF16, kind="ExternalInput")
    out = nc.dram_tensor("out", [T, DOUT], BF16, kind="ExternalOutput")

    with tile.TileContext(nc) as tc:
        with ExitStack() as ctx:
            persist = ctx.enter_context(tc.tile_pool(name="persist", bufs=1))
            QT_sb = persist.tile([128, FT, T], BF16)
            KT_sb = persist.tile([128, FT, T], BF16)
            VA_sb = persist.tile([128, NTOK, NH, HD + 1], BF16)
            CTX_sb = persist.tile([128, FT, T], BF16)
            bq_sb = persist.tile([128, FT], F32)
            bk_sb = persist.tile([128, FT], F32)
            bv_sb = persist.tile([128, F], F32)
            mask01_sb = persist.tile([128, 128], BF16)
            Wo_sb = persist.tile([128, FT, DOUT], BF16)

            # psum pools: pproj(2) + pS(2x2 banks) + pctx(2) = 8 banks exactly.
            ppool = ctx.enter_context(tc.tile_pool(name="pmain", bufs=2, space="PSUM"))
            pS = ctx.enter_context(tc.tile_pool(name="pS", bufs=2, space="PSUM"))
            pctxp = ctx.enter_context(tc.tile_pool(name="pctx", bufs=2, space="PSUM"))
            ptile = ctx.enter_context(tc.tile_pool(name="ptile", bufs=3))
            cnp = ctx.enter_context(tc.tile_pool(name="cn", bufs=4))
            rp = ctx.enter_context(tc.tile_pool(name="rp", bufs=4))

            with tc.tile_pool(name="wqkv", bufs=1) as wpool, \
                 tc.tile_pool(name="xin", bufs=2) as xpool, \
                 tc.tile_pool(name="osb", bufs=4) as osb:
                W_sb = {}
                for key in xdr:
                    wtile = wpool.tile([128, DT, F], FP8, tag="w" + key, name="W_" + key)
                    W_sb[key] = wtile

                def load_w(key):
                    wr = wdr[key][:].rearrange("(dt p) f -> p dt f", p=128)
                    nc.sync.dma_start(W_sb[key][:], wr[:])

                def load_x(nm, tj, halves=("h", "l"), eng=None):
                    ts = {}
                    for half in halves:
                        t_ = xpool.tile([128, DT, TQ], FP8, tag="xin" + nm + half)
                        xr = xdr[nm + half][:].rearrange("(dt p) t -> p dt t", p=128)
                        (eng or nc.gpsimd).dma_start(
                            t_[:], xr[:, :, tj * TQ:(tj + 1) * TQ])
                        ts[half] = t_
                    return ts

                def v_group(tj, c, vt):
                    tt = tj * NR + c
                    ps = ppool.tile([128, TQ], F32, tag="pproj")
                    psv = ps[:, :F]
                    for ti, (xh, wh) in enumerate((("h", "h"), ("l", "h"), ("h", "l"))):
                        for t in range(DP):
                            nc.tensor.matmul(
                                psv,
                                lhsT=vt[xh][:, 2 * t:2 * t + 2, c * 128:(c + 1) * 128],
                                rhs=W_sb["v" + wh][:, 2 * t:2 * t + 2, :],
                                start=(ti == 0 and t == 0),
                                stop=(ti == 2 and t == DP - 1),
                                perf_mode=DR)
                    for h in range(NH):
                        nc.vector.tensor_tensor(
                            VA_sb[:, tt, h, 0:HD],
                            psv[:, h * HD:(h + 1) * HD],
                            bv_sb[:, h * HD:(h + 1) * HD],
                            OP.add)

                def v_proj(tj, vt):
                    for c in range(NR):
                        v_group(tj, c, vt)

                def qk_group(which, tj, ft, xt):
                    nm, dst, bsb, sc = which
                    ps = ppool.tile([128, TQ], F32, tag="pproj")
                    for ti, (xh, wh) in enumerate((("h", "h"), ("l", "h"), ("h", "l"))):
                        for t in range(DP):
                            nc.tensor.matmul(
                                ps[:],
                                lhsT=W_sb[nm + wh][:, 2 * t:2 * t + 2, ft * 128:(ft + 1) * 128],
                                rhs=xt[xh][:, 2 * t:2 * t + 2, :],
                                start=(ti == 0 and t == 0),
                                stop=(ti == 2 and t == DP - 1),
                                perf_mode=DR)
                    nc.vector.tensor_scalar(
                        dst[:, ft, tj * TQ:(tj + 1) * TQ], ps[:],
                        sc, bsb[:, ft:ft + 1], OP.mult, OP.add)

                def qk_proj(which, tj, xt):
                    for ft in range(FT):
                        qk_group(which, tj, ft, xt)

                QSPEC = ("q", QT_sb, bq_sb, scale / 32.0)
                KSPEC = ("k", KT_sb, bk_sb, 1.0 / 32.0)

                def out_proj_tt(tt):
                    ot = osb.tile([128, DOUT], BF16, tag="ot")
                    for n in range(NO):
                        ps = ppool.tile([128, NCH], F32, tag="pproj")
                        for ft in range(FT):
                            nc.tensor.matmul(
                                ps[:],
                                lhsT=CTX_sb[:, ft, tt * 128:(tt + 1) * 128],
                                rhs=Wo_sb[:, ft, n * NCH:(n + 1) * NCH],
                                start=(ft == 0), stop=(ft == FT - 1))
                        nc.vector.tensor_copy(ot[:, n * NCH:(n + 1) * NCH], ps[:])
                    nc.sync.dma_start(out[tt * 128:(tt + 1) * 128, :], ot[:])

                def out_proj(tj):
                    for c in range(NR):
                        out_proj_tt(tj * NR + c)

                work_q = []   # FIFO of (kind, serial, closure) PE work bursts
                cn_tiles = {}     # tqblk-local -> ctx_n2 tile shared by a head pair

                def drain(k=None, upto_chain=None, kinds=None):
                    """Run queued bursts. k: at most k items. upto_chain:
                    run until no chain with serial <= upto_chain remains
                    (keeps tile-pool rotation safe). kinds: with upto_chain
                    unset, run until no item of these kinds remains."""
                    n = 0
                    while work_q:
                        if upto_chain is not None:
                            if not any(kd == "chain" and sr <= upto_chain
                                       for kd, sr, _ in work_q):
                                break
                        elif kinds is not None:
                            if not any(kd in kinds for kd, sr, _ in work_q):
                                break
                        elif k is not None and n >= k:
                            break
                        _, _, fn = work_q.pop(0)
                        fn()
                        n += 1

                def attention(h, tj):
                    serial = tj * NH + h
                    if h == 0:
                        # tj boundary: this tile's projections and all older
                        # chains must be issued first; deferred output
                        # projections may linger as later filler
                        drain(kinds=("chain", "proj"))
                    else:
                        drain(upto_chain=serial - 2)
                    """Scores + exp for head h of tile tj (swapped-PV layout).

                    Score pairs stream into 2-bank psum tiles, exp'd into a
                    ping-pong pt buffer [tk, key-block, tq]. The PV chains
                    (out[tq,65] = pt_block.T @ VA, accumulated over key
                    blocks) are queued and interleaved between the NEXT
                    head's score pairs so the PE never waits on exp. Each
                    chain ends with a per-partition reciprocal+normalize on
                    DVE into a head-pair staging tile that is DMA-transposed
                    into CTX once both heads have written it.
                    """
                    ft, po = h // HPF, (h % HPF) * HD
                    half = h % HPF
                    QhT = QT_sb[po:po + HD, ft, :]
                    KhT = KT_sb[po:po + HD, ft, :]
                    nblk = NR * tj + NR
                    npair = nblk // 2
                    ptb = ptile.tile([128, NTOK, TQ], BF16, tag="pt")

                    def blk_c0(i):
                        r = i - NR * tj
                        return 128 * r if r > 0 else 0

                    for pi in range(npair):
                        ps2 = pS.tile([128, 2, TQ], F32, tag="pS")
                        for jj in range(2):
                            i = 2 * pi + jj
                            c0 = blk_c0(i)
                            nc.tensor.matmul(
                                ps2[:, jj, c0:],
                                lhsT=KhT[:, i * 128:(i + 1) * 128],
                                rhs=QhT[:, tj * TQ + c0:(tj + 1) * TQ],
                                start=True, stop=True)
                        c0p = blk_c0(2 * pi)
                        nc.scalar.activation(
                            ptb[:, 2 * pi:2 * pi + 2, c0p:], ps2[:, :, c0p:], AF.Exp)
                        drain(k=1 + (len(work_q) > 16) + (len(work_q) > 40))

                    def make_chain(t, ptb=ptb, h=h, tj=tj, ft=ft, half=half):
                        def chain():
                            gt = NR * tj + t
                            # only slab gt's causally-invalid triangle overlaps
                            # this chain's column slice; zero it here (its exp
                            # finished a head ago, so this never stalls)
                            nc.vector.tensor_tensor(
                                ptb[:, gt, t * 128:(t + 1) * 128],
                                ptb[:, gt, t * 128:(t + 1) * 128],
                                mask01_sb[:], OP.mult)
                            pctxf = pctxp.tile([128, 512], F32, tag="pctx")
                            pctx = pctxf[:, :HD + 1]
                            for i in range(gt + 1):
                                nc.tensor.matmul(
                                    pctx[:],
                                    lhsT=ptb[:, i, t * 128:(t + 1) * 128],
                                    rhs=VA_sb[:, i, h, :],
                                    start=(i == 0), stop=(i == gt))
                            recip1 = rp.tile([128, 1], F32, tag="recip")
                            nc.vector.reciprocal(recip1[:], pctx[:, HD:HD + 1])
                            if half == 0:
                                cn = cnp.tile([128, 2 * HD], BF16, tag=f"cn{t}",
                                              name=f"cn{t}")
                                cn_tiles[t] = cn
                            else:
                                cn = cn_tiles[t]
                            nc.vector.tensor_scalar(
                                cn[:, half * HD:(half + 1) * HD], pctx[:, 0:HD],
                                1.0 / 32.0, recip1[:], OP.mult, OP.mult)
                            if half == 1:
                                tt = NR * tj + t
                                nc.sync.dma_start_transpose(
                                    CTX_sb[:, ft, tt * 128:(tt + 1) * 128], cn[:])
                        return chain

                    for t in range(NR):
                        work_q.append(("chain", serial, make_chain(t)))

                # prologue: DMAs in dependency-priority order (attention
                # starts on Q/K, so load those first; V before the PV chains
                # of head 0, which run a head later). x tiles are prefetched
                # a full tq tile ahead throughout.
                nc.sync.dma_start(mask01_sb[:], mask01[:])
                xcur = {"q": load_x("q", 0, ("h",))}
                load_w("qh")
                xcur["q"].update(load_x("q", 0, ("l",)))
                load_w("ql")
                nc.sync.dma_start(bq_sb[:], bq[:])
                xcur["k"] = load_x("k", 0, ("h",))
                load_w("kh")
                xcur["k"].update(load_x("k", 0, ("l",)))
                load_w("kl")
                nc.sync.dma_start(bk_sb[:], bk[:])
                xcur["v"] = load_x("v", 0, ("h",))
                load_w("vh")
                xcur["v"].update(load_x("v", 0, ("l",)))
                load_w("vl")
                nc.sync.dma_start(bv_sb[:], bv[:].to_broadcast([128, F]))
                nc.vector.memset(
                    VA_sb[:].rearrange("p a b c -> p (a b) c")[:, :, HD:HD + 1], 1.0)
                qk_proj(QSPEC, 0, xcur["q"])
                qk_proj(KSPEC, 0, xcur["k"])
                v_proj(0, xcur["v"])
                xnext = {nm: load_x(nm, 1) for nm in ("v", "q", "k")}

                for tj in range(NTQ):
                    for h in range(NH):
                        attention(h, tj)
                        if tj == 0 and h == 0:
                            # Wo isn't needed until the first deferred output
                            # projection; keep it off the critical DMA window
                            nc.sync.dma_start(
                                Wo_sb[:],
                                Wo[:].rearrange("(ft p) n -> p ft n", p=128))
                        if h == 5 and tj > 0:
                            # defer the oldest settled tile's output projection;
                            # at the last tile, also queue the previous one so
                            # the ACT-bound phase has PE filler work.
                            fr = [tj - 2] if tj > 1 else []
                            if tj == NTQ - 1:
                                fr.append(tj - 1)
                            for f_ in fr:
                                for c in range(NR):
                                    work_q.append(
                                        ("oproj", 0,
                                         lambda tt=f_ * NR + c: out_proj_tt(tt)))
                        if tj + 1 < NTQ:
                            if h == 1:
                                for c in range(NR):
                                    work_q.append(
                                        ("proj", 0,
                                         lambda c=c, vt=xnext["v"], t=tj + 1:
                                         v_group(t, c, vt)))
                            elif h == 3:
                                for ft in range(FT):
                                    work_q.append(
                                        ("proj", 0,
                                         lambda ft=ft, xt=xnext["q"], t=tj + 1:
                                         qk_group(QSPEC, t, ft, xt)))
                            elif h == 5:
                                for ft in range(FT):
                                    work_q.append(
                                        ("proj", 0,
                                         lambda ft=ft, xt=xnext["k"], t=tj + 1:
                                         qk_group(KSPEC, t, ft, xt)))
                        if h == 6 and tj + 2 < NTQ:
                            xnext = {nm: load_x(nm, tj + 2)
                                     for nm in ("v", "q", "k")}
                drain()
                out_proj(NTQ - 1)

    nc.compile()
    return nc


def make_mask():
    """mask01[p, f] = 0 where f < p else 1 (diagonal-block causal mask)."""
    p = np.arange(128)[:, None]
    f = np.arange(128)[None, :]
    m = np.where(f < p, np.float32(0.0), np.float32(1.0))
    return m.astype(ml_dtypes.bfloat16)


def _split8(x):
    hi = x.astype(ml_dtypes.float8_e4m3)
    lo = (x - hi.astype(np.float32)).astype(ml_dtypes.float8_e4m3)
    return hi, lo


def make_core_inputs(q_b, k_b, v_b, Wq, bq, Wk, bk, Wv, bv, Wo, fsl, scale):
    """Build the in_map for one core. fsl = feature slice for this core's heads."""
    F = fsl.stop - fsl.start
    FT = F // 128
    d = {}
    for nm, x in (("q", q_b), ("k", k_b), ("v", v_b)):
        hi, lo = _split8(np.ascontiguousarray(x.T))
        d[f"{nm}Th"], d[f"{nm}Tl"] = hi, lo
    # weights are scaled by 32 so the fp8 residual stays in normal range;
    # the kernel folds 1/32 back in the copyback / normalize constants
    for nm, W in (("q", Wq), ("k", Wk), ("v", Wv)):
        hi, lo = _split8(np.ascontiguousarray(W[:, fsl]) * np.float32(32.0))
        d[f"W{nm}h"], d[f"W{nm}l"] = hi, lo
    d["Wo"] = np.ascontiguousarray(Wo[fsl, :]).astype(ml_dtypes.bfloat16)
    d["bq"] = np.ascontiguousarray((bq[fsl] * scale).reshape(FT, 128).T)
    d["bk"] = np.ascontiguousarray(bk[fsl].reshape(FT, 128).T)
    d["bv"] = np.ascontiguousarray(bv[fsl].reshape(1, F) * np.float32(32.0))
    d["ones"] = np.ones((1, 1), np.float32)
    d["mask01"] = make_mask()
    return d


_CACHE = {}


def kernel(q, k, v, Wq, bq, Wk, bk, Wv, bv, Wo, bo, _trace=False):
    B, T, D = q.shape
    H, HD = 16, 64
    scale = np.float32(1.0 / np.sqrt(D))
    n_cores = 8
    gpb = n_cores // B            # head-groups per batch element (2)
    F = D // gpb                  # feature columns per core (512)

    key = (T, D, F)
    if key not in _CACHE:
        _CACHE[key] = build_mha_core(T=T, D=D, F=F, DOUT=D, HD=HD, TQ=512,
                                     scale=float(scale), num_devices=n_cores)
    nc = _CACHE[key]

    q = np.asarray(q, np.float32)
    k = np.asarray(k, np.float32)
    v = np.asarray(v, np.float32)
    in_maps = []
    for c in range(n_cores):
        b, g = c // gpb, c % gpb
        fsl = slice(g * F, (g + 1) * F)
        in_maps.append(make_core_inputs(
            q[b], k[b], v[b], Wq, bq, Wk, bk, Wv, bv, Wo, fsl, scale))

    res = run_bass_kernel_spmd(nc, in_maps, list(range(n_cores)), trace=_trace)
    out = np.zeros((B, T, D), np.float32)
    for c in range(n_cores):
        out[c // gpb] += np.asarray(res.results[c]["out"], np.float32)
    out += np.asarray(bo, np.float32)
    if _trace:
        kernel.last_exec_time_ns = res.exec_time_ns
    return out


# revision 61
# speedup vs baseline: 1.0631x; 1.0631x over previous
"""Trainium2 Bass kernel for causal multi-head attention.

Problem: B=4, T=2048, D=1024, H=16, HD=64, fp32, causal, scale=1/sqrt(D).

Sharding: 4-way batch x 2-way head-group over 8 cores. Core c=(b,g) computes
heads g*8..g*8+7 for batch b and returns the partial output projection
(contracted over its 512 context columns); the host sums the two partials
per batch element and adds bo.

Per-core dataflow:
  - Q/K/V projections run in fp8e4m3 DoubleRow mode (0.5 cycles/row, 256-deep
    contraction per instruction) using a hi+lo residual split of both the
    inputs and the weights: x@W ~= (xh+xl)@Wh + xh@Wl, which keeps the
    projection error at bf16 level while running ~2.7x faster than bf16.
    Weights are host-scaled by 32 so the fp8 residual stays in normal range
    (W ~ N(0,1/D) residuals are subnormal in e4m3 otherwise); 1/32 is folded
    back into the copyback / normalize constants. Host passes x^T pre-split
    into fp8 hi/lo pairs, so no on-device transposes or casts are needed.
  - Scores are computed transposed, S^T[tk_part, tq_free] (lhsT = K^T
    block, rhs = Q^T tile), in bf16; blocks above the causal diagonal are
    skipped entirely. Masking costs the PE nothing: each PV chain overlaps
    exactly one diagonal block's invalid triangle, which is zeroed by a
    single in-place DVE multiply inside the chain (its exp finished a head
    earlier, so the multiply never stalls anything).
  - Score blocks are exp'd in PAIRS: two 128x512 PSUM banks per scalar-engine
    instruction, halving the per-op overhead. The second slab of a diagonal
    pair reads some never-written PSUM lanes; the resulting garbage probs
    land in pt regions no consumer references.
  - PV runs in the SWAPPED orientation: out[tq,65] = pt_block^T @ VA,
    accumulated over key blocks (65-row matmuls are ~2x cheaper than the
    [65,tq] orientation and the tensor engine loads pt blocks as stationary
    weights for free). V is stored bf16 with a ones-column appended per head
    so the same accumulation yields the softmax denominator per PARTITION,
    which makes normalization a per-partition reciprocal + tensor_scalar on
    DVE - no partition-broadcast needed. The normalized [tok,128] tile of a
    HEAD PAIR is DMA-transposed (XBAR) back into the resident ctx^T buffer.
  - All PE work beyond the score stream (PV chains of the previous head,
    fp8 projection groups for the next tile, deferred output projections)
    lives in a FIFO work queue drained between score pairs, so the PE stays
    busy while the scalar engine paces the exp stream. x tiles are
    prefetched a full tq tile ahead; output projections are deferred two
    tiles so the ACT-bound last tile has PE filler work.
"""

import numpy as np
from contextlib import ExitStack

import ml_dtypes
import concourse.bass as bass
import concourse.tile as tile
from concourse import bacc
from concourse import mybir
from concourse.bass_utils import run_bass_kernel_spmd

F32 = mybir.dt.float32
F32R = mybir.dt.float32r
BF16 = mybir.dt.bfloat16
FP8 = mybir.dt.float8e4
AF = mybir.ActivationFunctionType
OP = mybir.AluOpType
DR = mybir.MatmulPerfMode.DoubleRow


def build_mha_core(T, D, F, DOUT, HD=64, TQ=512, scale=1.0, num_devices=1):
    """Build the per-core Bass program.

    T: tokens, D: model dim, F: feature columns owned by this core,
    DOUT: output projection width, HD: head dim, TQ: tq tile width.
    """
    NH = F // HD        # local heads
    DT = D // 128       # contraction tiles for projections
    DP = DT // 2        # DoubleRow pair count
    FT = F // 128       # feature 128-tiles
    NTOK = T // 128     # token 128-tiles
    NTQ = T // TQ       # tq tiles
    NR = TQ // 128      # 128-blocks per tq tile
    NCH = min(512, DOUT)
    NO = DOUT // NCH
    HPF = 128 // HD     # heads per feature tile

    nc = bacc.Bacc(None, target_bir_lowering=False, debug=False, num_devices=num_devices)

    xdr = {}
    for nm in ("q", "k", "v"):
        for half in ("h", "l"):
            xdr[nm + half] = nc.dram_tensor(f"{nm}T{half}", [D, T], FP8, kind="ExternalInput")
    wdr = {}
    for nm in ("q", "k", "v"):
        for half in ("h", "l"):
            wdr[nm + half] = nc.dram_tensor(f"W{nm}{half}", [D, F], FP8, kind="ExternalInput")
    Wo = nc.dram_tensor("Wo", [F, DOUT], BF16, kind="ExternalInput")
    bq = nc.dram_tensor("bq", [128, FT], F32, kind="ExternalInput")
    bk = nc.dram_tensor("bk", [128, FT], F32, kind="ExternalInput")
    bv = nc.dram_tensor("bv", [1, F], F32, kind="ExternalInput")
    ones = nc.dram_tensor("ones", [1, 1], BF16, kind="ExternalInput")
    mask01 = nc.dram_tensor("mask01", [128, 128], BF16, kind="ExternalInput")
    out = nc.dram_tensor("out", [T, DOUT], BF16, kind="ExternalOutput")

    with tile.TileContext(nc) as tc:
        with ExitStack() as ctx:
            persist = ctx.enter_context(tc.tile_pool(name="persist", bufs=1))
            QT_sb = persist.tile([128, FT, T], BF16)
            KT_sb = persist.tile([128, FT, T], BF16)
            VA_sb = persist.tile([128, NTOK, NH, HD + 1], BF16)
            CTX_sb = persist.tile([128, FT, T], BF16)
            bq_sb = persist.tile([128, FT], F32)
            bk_sb = persist.tile([128, FT], F32)
            bv_sb = persist.tile([128, F], F32)
            mask01_sb = persist.tile([128, 128], BF16)
            Wo_sb = persist.tile([128, FT, DOUT], BF16)

            # psum pools: pproj(2) + pS(2x2 banks) + pctx(2) = 8 banks exactly.
            ppool = ctx.enter_context(tc.tile_pool(name="pmain", bufs=2, space="PSUM"))
            pS = ctx.enter_context(tc.tile_pool(name="pS", bufs=2, space="PSUM"))
            pctxp = ctx.enter_context(tc.tile_pool(name="pctx", bufs=2, space="PSUM"))
            ptile = ctx.enter_context(tc.tile_pool(name="ptile", bufs=3))
            cnp = ctx.enter_context(tc.tile_pool(name="cn", bufs=4))
            rp = ctx.enter_context(tc.tile_pool(name="rp", bufs=4))

            with tc.tile_pool(name="wqkv", bufs=1) as wpool, \
                 tc.tile_pool(name="xin", bufs=2) as xpool, \
                 tc.tile_pool(name="osb", bufs=4) as osb:
                W_sb = {}
                for key in xdr:
                    wtile = wpool.tile([128, DT, F], FP8, tag="w" + key, name="W_" + key)
                    W_sb[key] = wtile

                def load_w(key):
                    wr = wdr[key][:].rearrange("(dt p) f -> p dt f", p=128)
                    nc.sync.dma_start(W_sb[key][:], wr[:])

                def load_x(nm, tj, halves=("h", "l"), eng=None):
                    ts = {}
                    for half in halves:
                        t_ = xpool.tile([128, DT, TQ], FP8, tag="xin" + nm + half)
                        xr = xdr[nm + half][:].rearrange("(dt p) t -> p dt t", p=128)
                        (eng or nc.gpsimd).dma_start(
                            t_[:], xr[:, :, tj * TQ:(tj + 1) * TQ])
                        ts[half] = t_
                    return ts

                def v_group(tj, c, vt):
                    tt = tj * NR + c
                    ps = ppool.tile([128, TQ], F32, tag="pproj")
                    psv = ps[:, :F]
                    for ti, (xh, wh) in enumerate((("h", "h"), ("l", "h"), ("h", "l"))):
                        for t in range(DP):
                            nc.tensor.matmul(
                                psv,
                                lhsT=vt[xh][:, 2 * t:2 * t + 2, c * 128:(c + 1) * 128],
                                rhs=W_sb["v" + wh][:, 2 * t:2 * t + 2, :],
                                start=(ti == 0 and t == 0),
                                stop=(ti == 2 and t == DP - 1),
                                perf_mode=DR)
                    for h in range(NH):
                        nc.vector.tensor_tensor(
                            VA_sb[:, tt, h, 0:HD],
                            psv[:, h * HD:(h + 1) * HD],
                            bv_sb[:, h * HD:(h + 1) * HD],
                            OP.add)

                def v_proj(tj, vt):
                    for c in range(NR):
                        v_group(tj, c, vt)

                def qk_group(which, tj, ft, xt):
                    nm, dst, bsb, sc = which
                    ps = ppool.tile([128, TQ], F32, tag="pproj")
                    # 2-term for Q/K: xh @ (Wh + Wl). The dropped x residual
                    # costs ~1% on the output (scores are small, so the abs
                    # score noise stays tiny); V keeps all 3 terms since its
                    # error passes straight through to the output.
                    for ti, (xh, wh) in enumerate((("h", "h"), ("h", "l"))):
                        for t in range(DP):
                            nc.tensor.matmul(
                                ps[:],
                                lhsT=W_sb[nm + wh][:, 2 * t:2 * t + 2, ft * 128:(ft + 1) * 128],
                                rhs=xt[xh][:, 2 * t:2 * t + 2, :],
                                start=(ti == 0 and t == 0),
                                stop=(ti == 1 and t == DP - 1),
                                perf_mode=DR)
                    nc.vector.tensor_scalar(
                        dst[:, ft, tj * TQ:(tj + 1) * TQ], ps[:],
                        sc, bsb[:, ft:ft + 1], OP.mult, OP.add)

                def qk_proj(which, tj, xt):
                    for ft in range(FT):
                        qk_group(which, tj, ft, xt)

                QSPEC = ("q", QT_sb, bq_sb, scale / 32.0)
                KSPEC = ("k", KT_sb, bk_sb, 1.0 / 32.0)

                def out_proj_tt(tt):
                    ot = osb.tile([128, DOUT], BF16, tag="ot")
                    for n in range(NO):
                        ps = ppool.tile([128, NCH], F32, tag="pproj")
                        for ft in range(FT):
                            nc.tensor.matmul(
                                ps[:],
                                lhsT=CTX_sb[:, ft, tt * 128:(tt + 1) * 128],
                                rhs=Wo_sb[:, ft, n * NCH:(n + 1) * NCH],
                                start=(ft == 0), stop=(ft == FT - 1))
                        nc.vector.tensor_copy(ot[:, n * NCH:(n + 1) * NCH], ps[:])
                    nc.sync.dma_start(out[tt * 128:(tt + 1) * 128, :], ot[:])

                def out_proj(tj):
                    for c in range(NR):
                        out_proj_tt(tj * NR + c)

                work_q = []   # FIFO of (kind, serial, closure) PE work bursts
                cn_tiles = {}     # tqblk-local -> ctx_n2 tile shared by a head pair

                def drain(k=None, upto_chain=None, kinds=None):
                    """Run queued bursts. k: at most k items. upto_chain:
                    run until no chain with serial <= upto_chain remains
                    (keeps tile-pool rotation safe). kinds: with upto_chain
                    unset, run until no item of these kinds remains."""
                    n = 0
                    while work_q:
                        if upto_chain is not None:
                            if not any(kd == "chain" and sr <= upto_chain
                                       for kd, sr, _ in work_q):
                                break
                        elif kinds is not None:
                            if not any(kd in kinds for kd, sr, _ in work_q):
                                break
                        elif k is not None and n >= k:
                            break
                        _, _, fn = work_q.pop(0)
                        fn()
                        n += 1

                def attention(h, tj):
                    serial = tj * NH + h
                    if h == 0:
                        # tj boundary: this tile's projections and all older
                        # chains must be issued first; deferred output
                        # projections may linger as later filler
                        drain(kinds=("chain", "proj"))
                    else:
                        drain(upto_chain=serial - 2)
                    """Scores + exp for head h of tile tj (swapped-PV layout).

                    Score pairs stream into 2-bank psum tiles, exp'd into a
                    ping-pong pt buffer [tk, key-block, tq]. The PV chains
                    (out[tq,65] = pt_block.T @ VA, accumulated over key
                    blocks) are queued and interleaved between the NEXT
                    head's score pairs so the PE never waits on exp. Each
                    chain ends with a per-partition reciprocal+normalize on
                    DVE into a head-pair staging tile that is DMA-transposed
                    into CTX once both heads have written it.
                    """
                    ft, po = h // HPF, (h % HPF) * HD
                    half = h % HPF
                    QhT = QT_sb[po:po + HD, ft, :]
                    KhT = KT_sb[po:po + HD, ft, :]
                    nblk = NR * tj + NR
                    npair = nblk // 2
                    ptb = ptile.tile([128, NTOK, TQ], BF16, tag="pt")

                    def blk_c0(i):
                        r = i - NR * tj
                        return 128 * r if r > 0 else 0

                    for pi in range(npair):
                        ps2 = pS.tile([128, 2, TQ], F32, tag="pS")
                        for jj in range(2):
                            i = 2 * pi + jj
                            c0 = blk_c0(i)
                            nc.tensor.matmul(
                                ps2[:, jj, c0:],
                                lhsT=KhT[:, i * 128:(i + 1) * 128],
                                rhs=QhT[:, tj * TQ + c0:(tj + 1) * TQ],
                                start=True, stop=True)
                        c0p = blk_c0(2 * pi)
                        nc.scalar.activation(
                            ptb[:, 2 * pi:2 * pi + 2, c0p:], ps2[:, :, c0p:], AF.Exp)
                        drain(k=1 + (len(work_q) > 16) + (len(work_q) > 40))

                    def make_chain(t, ptb=ptb, h=h, tj=tj, ft=ft, half=half):
                        def chain():
                            gt = NR * tj + t
                            # only slab gt's causally-invalid triangle overlaps
                            # this chain's column slice; zero it here (its exp
                            # finished a head ago, so this never stalls)
                            nc.vector.tensor_tensor(
                                ptb[:, gt, t * 128:(t + 1) * 128],
                                ptb[:, gt, t * 128:(t + 1) * 128],
                                mask01_sb[:], OP.mult)
                            pctxf = pctxp.tile([128, 512], F32, tag="pctx")
                            pctx = pctxf[:, :HD + 1]
                            for i in range(gt + 1):
                                nc.tensor.matmul(
                                    pctx[:],
                                    lhsT=ptb[:, i, t * 128:(t + 1) * 128],
                                    rhs=VA_sb[:, i, h, :],
                                    start=(i == 0), stop=(i == gt))
                            recip1 = rp.tile([128, 1], F32, tag="recip")
                            nc.vector.reciprocal(recip1[:], pctx[:, HD:HD + 1])
                            if half == 0:
                                cn = cnp.tile([128, 2 * HD], BF16, tag=f"cn{t}",
                                              name=f"cn{t}")
                                cn_tiles[t] = cn
                            else:
                                cn = cn_tiles[t]
                            nc.vector.tensor_scalar(
                                cn[:, half * HD:(half + 1) * HD], pctx[:, 0:HD],
                                1.0 / 32.0, recip1[:], OP.mult, OP.mult)
                            if half == 1:
                                tt = NR * tj + t
                                nc.sync.dma_start_transpose(
                                    CTX_sb[:, ft, tt * 128:(tt + 1) * 128], cn[:])
                        return chain

                    for t in range(NR):
                        work_q.append(("chain", serial, make_chain(t)))

                # prologue: DMAs in dependency-priority order (attention
                # starts on Q/K, so load those first; V before the PV chains
                # of head 0, which run a head later). x tiles are prefetched
                # a full tq tile ahead throughout.
                nc.sync.dma_start(mask01_sb[:], mask01[:])
                xcur = {"q": load_x("q", 0, ("h",))}
                load_w("qh")
                load_w("ql")
                nc.sync.dma_start(bq_sb[:], bq[:])
                xcur["k"] = load_x("k", 0, ("h",))
                load_w("kh")
                load_w("kl")
                nc.sync.dma_start(bk_sb[:], bk[:])
                xcur["v"] = load_x("v", 0, ("h",))
                load_w("vh")
                xcur["v"].update(load_x("v", 0, ("l",)))
                load_w("vl")
                nc.sync.dma_start(bv_sb[:], bv[:].to_broadcast([128, F]))
                nc.vector.memset(
                    VA_sb[:].rearrange("p a b c -> p (a b) c")[:, :, HD:HD + 1], 1.0)
                qk_proj(QSPEC, 0, xcur["q"])
                qk_proj(KSPEC, 0, xcur["k"])
                v_proj(0, xcur["v"])
                xnext = {nm: load_x(nm, 1, ("h", "l") if nm == "v" else ("h",))
                         for nm in ("v", "q", "k")}

                for tj in range(NTQ):
                    for h in range(NH):
                        attention(h, tj)
                        if tj == 0 and h == 0:
                            # Wo isn't needed until the first deferred output
                            # projection; keep it off the critical DMA window
                            nc.sync.dma_start(
                                Wo_sb[:],
                                Wo[:].rearrange("(ft p) n -> p ft n", p=128))
                        if h == 5 and tj > 0:
                            # defer the oldest settled tile's output projection;
                            # at the last tile, also queue the previous one so
                            # the ACT-bound phase has PE filler work.
                            fr = [tj - 2] if tj > 1 else []
                            if tj == NTQ - 1:
                                fr.append(tj - 1)
                            for f_ in fr:
                                for c in range(NR):
                                    work_q.append(
                                        ("oproj", 0,
                                         lambda tt=f_ * NR + c: out_proj_tt(tt)))
                        if tj + 1 < NTQ:
                            if h == 1:
                                for c in range(NR):
                                    work_q.append(
                                        ("proj", 0,
                                         lambda c=c, vt=xnext["v"], t=tj + 1:
                                         v_group(t, c, vt)))
                            elif h == 3:
                                for ft in range(FT):
                                    work_q.append(
                                        ("proj", 0,
                                         lambda ft=ft, xt=xnext["q"], t=tj + 1:
                                         qk_group(QSPEC, t, ft, xt)))
                            elif h == 5:
                                for ft in range(FT):
                                    work_q.append(
                                        ("proj", 0,
                                         lambda ft=ft, xt=xnext["k"], t=tj + 1:
                                         qk_group(KSPEC, t, ft, xt)))
                        if h == 6 and tj + 2 < NTQ:
                            xnext = {nm: load_x(nm, tj + 2,
                                                ("h", "l") if nm == "v" else ("h",))
                                     for nm in ("v", "q", "k")}
                drain()
                out_proj(NTQ - 1)

    nc.compile()
    return nc


def make_mask():
    """mask01[p, f] = 0 where f < p else 1 (diagonal-block causal mask)."""
    p = np.arange(128)[:, None]
    f = np.arange(128)[None, :]
    m = np.where(f < p, np.float32(0.0), np.float32(1.0))
    return m.astype(ml_dtypes.bfloat16)


def _split8(x):
    hi = x.astype(ml_dtypes.float8_e4m3)
    lo = (x - hi.astype(np.float32)).astype(ml_dtypes.float8_e4m3)
    return hi, lo


def make_core_inputs(q_b, k_b, v_b, Wq, bq, Wk, bk, Wv, bv, Wo, fsl, scale):
    """Build the in_map for one core. fsl = feature slice for this core's heads."""
    F = fsl.stop - fsl.start
    FT = F // 128
    d = {}
    for nm, x in (("q", q_b), ("k", k_b), ("v", v_b)):
        hi, lo = _split8(np.ascontiguousarray(x.T))
        d[f"{nm}Th"], d[f"{nm}Tl"] = hi, lo
    # weights are scaled by 32 so the fp8 residual stays in normal range;
    # the kernel folds 1/32 back in the copyback / normalize constants
    for nm, W in (("q", Wq), ("k", Wk), ("v", Wv)):
        hi, lo = _split8(np.ascontiguousarray(W[:, fsl]) * np.float32(32.0))
        d[f"W{nm}h"], d[f"W{nm}l"] = hi, lo
    d["Wo"] = np.ascontiguousarray(Wo[fsl, :]).astype(ml_dtypes.bfloat16)
    d["bq"] = np.ascontiguousarray((bq[fsl] * scale).reshape(FT, 128).T)
    d["bk"] = np.ascontiguousarray(bk[fsl].reshape(FT, 128).T)
    d["bv"] = np.ascontiguousarray(bv[fsl].reshape(1, F) * np.float32(32.0))
    d["ones"] = np.ones((1, 1), np.float32)
    d["mask01"] = make_mask()
    return d


_CACHE = {}


def kernel(q, k, v, Wq, bq, Wk, bk, Wv, bv, Wo, bo, _trace=False):
    B, T, D = q.shape
    H, HD = 16, 64
    scale = np.float32(1.0 / np.sqrt(D))
    n_cores = 8
    gpb = n_cores // B            # head-groups per batch element (2)
    F = D // gpb                  # feature columns per core (512)

    key = (T, D, F)
    if key not in _CACHE:
        _CACHE[key] = build_mha_core(T=T, D=D, F=F, DOUT=D, HD=HD, TQ=512,
                                     scale=float(scale), num_devices=n_cores)
    nc = _CACHE[key]

    q = np.asarray(q, np.float32)
    k = np.asarray(k, np.float32)
    v = np.asarray(v, np.float32)
    in_maps = []
    for c in range(n_cores):
        b, g = c // gpb, c % gpb
        fsl = slice(g * F, (g + 1) * F)
        in_maps.append(make_core_inputs(
            q[b], k[b], v[b], Wq, bq, Wk, bk, Wv, bv, Wo, fsl, scale))

    res = run_bass_kernel_spmd(nc, in_maps, list(range(n_cores)), trace=_trace)
    out = np.zeros((B, T, D), np.float32)
    for c in range(n_cores):
        out[c // gpb] += np.asarray(res.results[c]["out"], np.float32)
    out += np.asarray(bo, np.float32)
    if _trace:
        kernel.last_exec_time_ns = res.exec_time_ns
    return out
